# revision 8
# baseline (speedup 1.0000x reference)
"""Causal multi-head attention (32 heads, seq=128, d_model=4096) on 8 TRN2 cores.

Sharding: tensor-parallel over heads. Core c owns heads 4c..4c+3, i.e. rows
512c:512(c+1) of Q/K/V and columns 512c:512(c+1) of O. Each core computes its
partial output O_c @ att_c as out^T (128, 4096); the host sums the 8 partials
and transposes back.

The kernel is DMA-bound (~36MB of weight traffic per core at ~360 GB/s), so
the structure is a single saturated HBM stream x -> Q^T -> K^T -> V^T -> O^T
with all compute hidden underneath:

- All weight shards are host-packed into [128, 16384] layouts whose SBUF
  partition rows are 16KB contiguous in DRAM, so every DMA descriptor is a
  full 8-16KB line (vs 2KB rows for naive tiles) and each weight needs only
  8 dma_start issues (the sync engine's ~620ns/issue no longer bounds the
  stream rate).
- Big matmuls run in float32r (PE 1 cycle/row vs 4 for exact fp32, for
  output free dim >=256). The BIR verifier requires fp32r operands to be
  *produced* as fp32r, so weights are declared fp32r from DRAM onward and
  the attention output is retyped via a tiny SBUF->SBUF DMA. fp32r adds
  ~2e-4 relative error; the harness gate is 2e-2.
- Q/K/V stream buffers are recycled (V^T reuses Q^T's SBUF, O^T reuses
  K^T's) so the working set fits in SBUF.
- Output stores issue from the gpsimd engine so a store waiting on compute
  never head-of-line blocks the weight stream on the sync engine's queue.
"""

import math
import sys

import numpy as np

sys.path.insert(0, "/opt/trn_rl_repo")

import concourse.bacc as bacc
import concourse.bass as bass
import concourse.mybir as mybir
import concourse.tile as tile
from concourse.bass import ts
from concourse.bass_utils import run_bass_kernel_spmd
from concourse.masks import make_identity

P = 128
DM = 4096          # d_model
SEQ = 128
DK = 128           # head dim
NCORES = 8
HPC = 4            # heads per core
OW = HPC * DK      # 512: per-core projection width
KT = DM // P       # 32 contraction tiles
NCHUNK = DM // OW  # 8 output chunks
WCOLS = KT * OW    # 16384: packed weight free size
F32 = mybir.dt.float32
F32R = mybir.dt.float32r
SCALE = 1.0 / math.sqrt(DK)


def build_nc():
    nc = bacc.Bacc("TRN2", target_bir_lowering=False, debug=False)

    # Host-packed weight streams: partition p, col it*512+j holds W[128it+p, j]
    # (for ot: col (c*4+h)*512+j holds O^T[128h+p, 512c+j]).
    qt = nc.dram_tensor("qt", (P, WCOLS), F32R, kind="ExternalInput")
    kt = nc.dram_tensor("kt", (P, WCOLS), F32R, kind="ExternalInput")
    vt = nc.dram_tensor("vt", (P, WCOLS), F32R, kind="ExternalInput")
    ot = nc.dram_tensor("ot", (P, WCOLS), F32R, kind="ExternalInput")
    xt = nc.dram_tensor("xt", (P, DM), F32R, kind="ExternalInput")
    cmask_d = nc.dram_tensor("cmask", (P, P), F32, kind="ExternalInput")
    out = nc.dram_tensor("out", (SEQ, DM), F32, kind="ExternalOutput")

    with tile.TileContext(nc) as tc:
        with (
            tc.tile_pool(name="const", bufs=1) as cpool,
            tc.tile_pool(name="xtp", bufs=1) as xtp,
            tc.tile_pool(name="big", bufs=1) as big,
            tc.tile_pool(name="sb", bufs=1) as sb,
            tc.tile_pool(name="attn", bufs=2) as attnp,
            tc.tile_pool(name="attr", bufs=4) as attrp,
            tc.tile_pool(name="outp", bufs=3) as outp,
        ):
            # ---- The HBM stream, in consumption order. 2MB per weight
            # chunk: 16KB contiguous DRAM per partition row. The one DGE
            # ring executes in issue order, so arrival order == this order.
            xt_sb = xtp.tile([P, DM], F32R)
            nc.sync.dma_start(xt_sb, xt[:, :])
            qt_sb = big.tile([P, WCOLS], F32R, tag="w0")
            for j in range(NCHUNK):
                nc.sync.dma_start(qt_sb[:, ts(j, WCOLS // NCHUNK)],
                                  qt[:, ts(j, WCOLS // NCHUNK)])
            cmask = cpool.tile([P, P], F32)
            nc.sync.dma_start(cmask, cmask_d[:, :])
            ident = cpool.tile([P, P], F32)
            make_identity(nc, ident)
            kt_sb = big.tile([P, WCOLS], F32R, tag="w1")
            for j in range(NCHUNK):
                nc.sync.dma_start(kt_sb[:, ts(j, WCOLS // NCHUNK)],
                                  kt[:, ts(j, WCOLS // NCHUNK)])

            # ---- Phase 1: projections, in stream-arrival order (all q,
            # then k, then v) so the in-order PE queue never head-of-line
            # blocks on a later stream while earlier data sits ready.
            with tc.tile_pool(name="psA", bufs=1, space="PSUM") as psA:
                q_ps = psA.tile([P, OW], F32, tag="q")
                k_ps = psA.tile([P, OW], F32, tag="k")
                v_ps = psA.tile([P, OW], F32, tag="v")
                for it in range(KT):
                    nc.tensor.matmul(q_ps, xt_sb[:, ts(it, SEQ)],
                                     qt_sb[:, ts(it, OW)],
                                     start=it == 0, stop=it == KT - 1)
                # fold 1/sqrt(dk) into q while copying out of PSUM
                q_sb = sb.tile([P, OW], F32, tag="q_sb")
                nc.vector.tensor_scalar_mul(q_sb, q_ps, SCALE)

                for it in range(KT):
                    nc.tensor.matmul(k_ps, xt_sb[:, ts(it, SEQ)],
                                     kt_sb[:, ts(it, OW)],
                                     start=it == 0, stop=it == KT - 1)
                k_sb = sb.tile([P, OW], F32, tag="k_sb")
                nc.vector.tensor_copy(k_sb, k_ps)

                # V^T recycles Q^T's buffer: the sync engine parks on
                # q-proj completion (~30us) before issuing, while the ring
                # is still busy with K^T — no DMA idle.
                vt_sb = big.tile([P, WCOLS], F32R, tag="w0")
                for j in range(NCHUNK):
                    nc.sync.dma_start(vt_sb[:, ts(j, WCOLS // NCHUNK)],
                                      vt[:, ts(j, WCOLS // NCHUNK)])
                for it in range(KT):
                    nc.tensor.matmul(v_ps, xt_sb[:, ts(it, SEQ)],
                                     vt_sb[:, ts(it, OW)],
                                     start=it == 0, stop=it == KT - 1)
                v_sb = sb.tile([P, OW], F32, tag="v_sb")
                nc.vector.tensor_copy(v_sb, v_ps)

            # O^T recycles K^T's buffer (k-proj done ~53us, ring busy with
            # V^T until ~75us). Chunk c's 4 head-tiles are contiguous.
            ot_sb = big.tile([P, WCOLS], F32R, tag="w1")
            for j in range(NCHUNK):
                nc.sync.dma_start(ot_sb[:, ts(j, WCOLS // NCHUNK)],
                                  ot[:, ts(j, WCOLS // NCHUNK)])

            # ---- Phase 2: per-head causal attention (overlaps V^T/O^T
            # streams). 1/rowsum is folded into p^T before the att matmul
            # so att_ps is final and can be retyped straight to fp32r.
            att_r = []
            with (
                tc.tile_pool(name="psC", bufs=2, space="PSUM") as psC,
                tc.tile_pool(name="psB", bufs=1, space="PSUM") as psB,
                tc.tile_pool(name="psS", bufs=2, space="PSUM") as psS,
            ):
                for h in range(HPC):
                    qT_ps = psB.tile([P, P], F32, tag="tq")
                    nc.tensor.transpose(qT_ps, q_sb[:, ts(h, DK)], ident)
                    qT_sb = attnp.tile([P, P], F32, tag="qT")
                    nc.vector.tensor_copy(qT_sb, qT_ps)
                    kT_ps = psB.tile([P, P], F32, tag="tk")
                    nc.tensor.transpose(kT_ps, k_sb[:, ts(h, DK)], ident)
                    kT_sb = attnp.tile([P, P], F32, tag="kT")
                    nc.vector.tensor_copy(kT_sb, kT_ps)

                    # scores[sq, sk] = q_h @ k_h^T  (1/sqrt(dk) folded into q)
                    sc_ps = psS.tile([P, P], F32, tag="sc")
                    nc.tensor.matmul(sc_ps, qT_sb, kT_sb,
                                     start=True, stop=True)
                    # causal mask (keep sk >= sq) and softmax; scores*scale
                    # is bounded (~|10|) so the single-exp softmax without
                    # max-subtraction is numerically safe here.
                    masked = attnp.tile([P, P], F32, tag="masked")
                    nc.vector.tensor_add(masked, sc_ps, cmask)
                    e = attnp.tile([P, P], F32, tag="e")
                    rowsum = attnp.tile([P, 1], F32, tag="rowsum")
                    nc.scalar.activation(e, masked,
                                         mybir.ActivationFunctionType.Exp,
                                         accum_out=rowsum)
                    recip = attnp.tile([P, 1], F32, tag="recip")
                    nc.vector.reciprocal(recip, rowsum)
                    p_sb = attnp.tile([P, P], F32, tag="p")
                    nc.vector.tensor_scalar_mul(p_sb, e, recip)

                    pT_ps = psB.tile([P, P], F32, tag="pt")
                    nc.tensor.transpose(pT_ps, p_sb, ident)
                    pT_sb = attnp.tile([P, P], F32, tag="pT")
                    nc.vector.tensor_copy(pT_sb, pT_ps)

                    att_ps = psB.tile([P, P], F32, tag="at")
                    nc.tensor.matmul(att_ps, pT_sb, v_sb[:, ts(h, DK)],
                                     start=True, stop=True)
                    a_sb = attnp.tile([P, P], F32, tag="a")
                    nc.vector.tensor_copy(a_sb, att_ps)
                    # retype to fp32r for the out-phase matmuls (tiny
                    # SBUF->SBUF DMA; gpsimd issue keeps sync's queue clean)
                    a_r = attrp.tile([P, P], F32R, tag="ar")
                    nc.gpsimd.dma_start(a_r, a_sb.bitcast(F32R))
                    att_r.append(a_r)

                # ---- Phase 3: out^T[dk, dm-chunk] = sum_h att_h^T @ O^T,
                # paced by the O^T stream; stores issue from gpsimd.
                for c in range(NCHUNK):
                    o_ps = psC.tile([P, OW], F32, tag="o")
                    for h in range(HPC):
                        nc.tensor.matmul(o_ps, att_r[h],
                                         ot_sb[:, ts(c * HPC + h, OW)],
                                         start=h == 0, stop=h == HPC - 1)
                    o_sb = outp.tile([P, OW], F32, tag="o_sb")
                    nc.vector.tensor_copy(o_sb, o_ps)
                    nc.gpsimd.dma_start(out[:, ts(c, OW)], o_sb)

    nc.compile()
    return nc


def make_in_maps(Q, K, V, O, x):
    Q = np.ascontiguousarray(np.asarray(Q, dtype=np.float32))
    K = np.ascontiguousarray(np.asarray(K, dtype=np.float32))
    V = np.ascontiguousarray(np.asarray(V, dtype=np.float32))
    O = np.ascontiguousarray(np.asarray(O, dtype=np.float32))
    x = np.ascontiguousarray(np.asarray(x, dtype=np.float32))
    # xt[p, it*128 + s] = x[s, it*128 + p]: contiguous 16KB SBUF rows
    xt = np.ascontiguousarray(
        x.T.reshape(KT, P, SEQ).transpose(1, 0, 2).reshape(P, DM)
    )
    sq = np.arange(SEQ)[:, None]
    sk = np.arange(SEQ)[None, :]
    cmask = np.where(sk >= sq, 0.0, -1e30).astype(np.float32)

    def pack_w(wt):  # (4096, 512) -> (128, 16384), row-contiguous stream
        return np.ascontiguousarray(
            wt.reshape(KT, P, OW).transpose(1, 0, 2).reshape(P, WCOLS)
        )

    def pack_o(otr):  # (512, 4096) -> (128, 16384), chunk-major head tiles
        return np.ascontiguousarray(
            otr.reshape(HPC, P, NCHUNK, OW).transpose(1, 2, 0, 3)
            .reshape(P, WCOLS)
        )

    in_maps = []
    for c in range(NCORES):
        sl = slice(c * OW, (c + 1) * OW)
        in_maps.append(
            {
                "qt": pack_w(np.ascontiguousarray(Q[sl].T)),
                "kt": pack_w(np.ascontiguousarray(K[sl].T)),
                "vt": pack_w(np.ascontiguousarray(V[sl].T)),
                "ot": pack_o(np.ascontiguousarray(O[:, sl].T)),
                "xt": xt,
                "cmask": cmask,
            }
        )
    return in_maps


_NC_CACHE = {}


def _get_nc():
    if "nc" not in _NC_CACHE:
        _NC_CACHE["nc"] = build_nc()
    return _NC_CACHE["nc"]


def kernel(Q, K, V, O, x, _trace=False):
    nc = _get_nc()
    in_maps = make_in_maps(Q, K, V, O, x)
    res = run_bass_kernel_spmd(
        nc, in_maps, core_ids=list(range(NCORES)), trace=_trace
    )
    acc = np.zeros((SEQ, DM), dtype=np.float64)
    for c in range(NCORES):
        acc += res.results[c]["out"].astype(np.float64)
    outT = acc.astype(np.float32)
    if _trace:
        kernel.last_exec_time_ns = res.exec_time_ns
        kernel.last_results = res
    return np.ascontiguousarray(outT.T)
